# revision 8
# baseline (speedup 1.0000x reference)
"""CompensatedSparseLinear on 8 TRN2 NeuronCores.

out[b,s,o] = sum_i x[b,s,i] * (W[o,i] + delta[o,i]) + b[o]

The sparse COO delta is folded into W on the host (scatter-add), leaving a
dense matmul: out2d = x2d @ W_eff^T + b with x2d [8192, 4096], W_eff [4096, 4096].

Sharding: data-parallel along the 8192 batch*seq rows — 1024 rows per core,
W_eff/b replicated. No collectives; host concatenates the output shards.

Per-core device kernel (out^T layout — out_features on PSUM partitions):
  outT[nt*128+ni, m] = sum_k W_eff[nt*128+ni, k] * x[m, k] + b[nt*128+ni]
  - operands in bfloat16 (fp32 PSUM accumulate): same 1 cyc/row PE rate as
    float32r but half the DMA traffic and SBUF footprint; rel absmax err
    ~2e-3 vs the 2e-2 gate (measured, vs 1.1e-4 for fp32r)
  - x^T shard resident in SBUF as 32 per-k tiles [ki, m] (64 KB/partition
    total): per-chunk DMA dependency granularity, so the first matmul
    waits only on its own k-chunk (~5 us into the cold run), not on the
    whole 8 MB x stream (~25 us) — matters for the graded single-shot
  - W_eff^T streamed per n-tile as [ki, (k, ni)] blocks (8 KB/partition,
    4-deep buffered), 4 DMAs per n-tile from contiguous host-pretiled DRAM
  - bias added via ScalarE activation(Identity, bias) — per-partition bias

Measured (in-NEFF For_i repeat differencing, R=4 vs 2004, min-of-5,
device-resident inputs): 650 us/iter vs 945 us for the fp32r baseline
under the identical protocol. Absolute numbers drift +10-25% as the
shared device heats over a session; compare variants back-to-back.

Optimization notes (2026-08-08 session, for future iterations):
- PE floor is 437 us/core (2048 matmuls x 512 rows @ 2.4 GHz, 1 cyc/row
  for bf16 = fp32r). The ~210 us gap is ~104 ns per matmul INSTRUCTION:
  it is insensitive to W streaming (removing all W DMA saved only 28 us)
  and to stationary-load amortization (TP2xDP4, same instr count, half
  the loads: 682 us — no gain). DMA total is only 213 us fully exposed.
- Fewer/bigger matmuls blocked: MB=1024 (2 PSUM banks) rejected by
  walrus codegen. Note compile path passes --enable-ldw-opt=false
  (hardcoded in concourse/bass_utils.py).
- fp8e4 2-pass (x split hi+lo, W*64 in fp8, shared-scale PSUM accum)
  measures rel absmax 1.74e-2 / rel rms 2.2e-2 on the real inputs via
  host numpy sim — too close to the 2e-2 gate to ship. 3-pass fixes
  error but needs 3072 DR instrs ~= no win at 104 ns/instr overhead.
- 2-deep W prefetch + out-DMA on gpsimd (SWDGE): 666 vs 718 us for the
  1-deep/SP-queue version, measured back-to-back. DVE cannot trigger
  HWDGE DMAs (SP/Act/gpsimd only).
- MEASUREMENT CAVEAT: in back-to-back bench pairs the FIRST kernel
  measures ~666-670 and the SECOND ~710-720 regardless of which is
  which (thermal ordering bias). v5 (xs bufs=2, 3-deep prefetch,
  WCH=2) = v4 within this noise; all post-bf16 scheduling deltas are
  <=1-2% once order is controlled. Use ABAB interleaved pairs or
  order-reversed repeats to resolve differences below ~5%.
- Naive DoubleRow matmul (lhsT [128,256] slice, rhs [128,1024] slice,
  flat 2D APs) builds+compiles but hard-crashes the exec unit
  (NRT_EXEC_UNIT_UNRECOVERABLE) — packing needs the real [128,2,128]
  interleave (or DoubleRowSwInterleave) before trying again.
"""

import numpy as np
import ml_dtypes

import concourse.bacc as bacc
import concourse.tile as tile
import concourse.mybir as mybir
from concourse.bass_utils import run_bass_kernel_spmd

# Problem shape (hardcoded — harness contract)
B, S, D_IN, D_OUT = 4, 2048, 4096, 4096
N_CORES = 8
M_TOT = B * S              # 8192 rows
M = M_TOT // N_CORES       # 1024 rows per core
KT = D_IN // 128           # 32 k tiles
NT = D_OUT // 128          # 32 n tiles
MB = 512                   # moving free-dim per matmul (PSUM bank)
NMB = M // MB              # 2 m-blocks per core
WCH = 4                    # DMA chunks per n-tile W block (alternating SP/Act queues)
W_BUFS = 4
O_BUFS = 3

_MM_DT = mybir.dt.bfloat16
_NP_MM = ml_dtypes.bfloat16

_CACHE: dict = {}
LAST = {"exec_time_ns": None}


def _build(repeat=1):
    nc = bacc.Bacc("TRN2", target_bir_lowering=False, debug=False)

    # xT[ki, k, m] = x_shard[m, k*128+ki]
    xT = nc.declare_dram_parameter("xT", [128, KT, M], _MM_DT, isOutput=False)
    # wt[nt, ki, k, ni] = W_eff[nt*128+ni, k*128+ki]
    wt = nc.declare_dram_parameter("wt", [NT, 128, KT, 128], _MM_DT, isOutput=False)
    # bias[ni, nt] = b[nt*128+ni]
    bias = nc.declare_dram_parameter("bias", [128, NT], mybir.dt.float32, isOutput=False)
    # outT[nt, ni, m]
    outT = nc.declare_dram_parameter("outT", [NT, 128, M], mybir.dt.float32, isOutput=True)

    with tile.TileContext(nc) as tc:
        with (
            tc.tile_pool(name="xp", bufs=KT) as xp,
            tc.tile_pool(name="bp", bufs=1) as bp,
            tc.tile_pool(name="wp", bufs=W_BUFS) as wp,
            tc.tile_pool(name="ps", bufs=8, space="PSUM") as ps,
            tc.tile_pool(name="op", bufs=O_BUFS) as op,
        ):
            def body(_iv=None):
                kc = KT // WCH  # k-tiles per W DMA chunk

                def load_w(nt):
                    # W block for this n-tile: [ki, (k, ni)] — 8 KB/partition.
                    # Chunks alternate between the two HWDGE trigger queues
                    # (qSPDynamicHW / qActDynamicHW) to double DMA issue BW.
                    w = wp.tile([128, KT * 128], _MM_DT, tag="w", name=f"w_{nt}")
                    for j in range(WCH):
                        eng = nc.sync if j % 2 == 0 else nc.scalar
                        eng.dma_start(
                            w[:, j * kc * 128 : (j + 1) * kc * 128],
                            wt[nt, :, j * kc : (j + 1) * kc, :],
                        )
                    return w

                bs = bp.tile([128, NT], mybir.dt.float32, name="bs")
                nc.sync.dma_start(bs[:], bias[:])

                # Cold-start critical path: the first matmul needs w0 and
                # xs[0] only. Issue w0 and the first x chunks before w1 so
                # the PE starts ~2 us earlier on the graded single shot.
                w_fifo = [load_w(0)]

                # resident x^T shard as KT per-k tiles (2 KB/partition each):
                # dependency granularity = one DMA chunk, not the whole shard
                xs = []

                def load_x(k):
                    xk = xp.tile([128, M], _MM_DT, tag="xs", name=f"xs_{k}")
                    eng = nc.sync if k % 2 == 0 else nc.scalar
                    eng.dma_start(xk[:], xT[:, k, :])
                    xs.append(xk)

                for k in range(4):
                    load_x(k)
                w_fifo.append(load_w(1))
                for k in range(4, KT):
                    load_x(k)

                for nt in range(NT):
                    # keep W prefetch 2 n-tiles ahead of the PE
                    w = w_fifo.pop(0)
                    if nt + 2 < NT:
                        w_fifo.append(load_w(nt + 2))
                    accs = [
                        ps.tile([128, MB], mybir.dt.float32, tag="ps", name=f"acc_{nt}_{mb}")
                        for mb in range(NMB)
                    ]
                    for k in range(KT):
                        for mb in range(NMB):
                            nc.tensor.matmul(
                                accs[mb][:],
                                w[:, k * 128 : (k + 1) * 128],
                                xs[k][:, mb * MB : (mb + 1) * MB],
                                start=(k == 0),
                                stop=(k == KT - 1),
                            )
                    o = op.tile([128, M], mybir.dt.float32, tag="o", name=f"o_{nt}")
                    for mb in range(NMB):
                        nc.scalar.activation(
                            o[:, mb * MB : (mb + 1) * MB],
                            accs[mb][:],
                            mybir.ActivationFunctionType.Identity,
                            bias=bs[:, nt : nt + 1],
                        )
                    # out DMA via gpsimd SWDGE: keeps the SP/Act HWDGE queues free for W prefetch
                    nc.gpsimd.dma_start(outT[nt], o[:])

            if repeat == 1:
                body()
            else:
                with tc.For_i(0, repeat, 1) as _i:
                    body(_i)

    nc.compile()
    return nc


def make_in_maps(x, W, b, delta_vals, delta_rows, delta_cols):
    x = np.asarray(x, dtype=np.float32)
    W = np.asarray(W, dtype=np.float32)
    b = np.asarray(b, dtype=np.float32)

    # Fold sparse delta into W (duplicate coords sum)
    W_eff = W.copy()
    np.add.at(
        W_eff,
        (np.asarray(delta_rows), np.asarray(delta_cols)),
        np.asarray(delta_vals, dtype=np.float32),
    )

    # wt[nt, ki, k, ni] = W_eff[nt*128+ni, k*128+ki]
    w_tiles = np.ascontiguousarray(
        W_eff.reshape(NT, 128, KT, 128).transpose(0, 3, 2, 1)
    ).astype(_NP_MM)
    bias_t = np.ascontiguousarray(b.reshape(NT, 128).T)

    x2d = x.reshape(M_TOT, D_IN)
    in_maps = []
    for c in range(N_CORES):
        shard = x2d[c * M : (c + 1) * M]  # [m, (k ki)]
        xT_c = np.ascontiguousarray(
            shard.reshape(M, KT, 128).transpose(2, 1, 0)
        ).astype(_NP_MM)
        in_maps.append({"xT": xT_c, "wt": w_tiles, "bias": bias_t})
    return in_maps


def kernel(x, W, b, delta_vals, delta_rows, delta_cols):
    in_maps = make_in_maps(x, W, b, delta_vals, delta_rows, delta_cols)

    if "nc" not in _CACHE:
        _CACHE["nc"] = _build()
    nc = _CACHE["nc"]

    res = run_bass_kernel_spmd(nc, in_maps, list(range(N_CORES)))
    LAST["exec_time_ns"] = res.exec_time_ns

    out2d = np.empty((M_TOT, D_OUT), dtype=np.float32)
    for c in range(N_CORES):
        outT_c = res.results[c]["outT"].reshape(D_OUT, M)  # [4096, 1024]
        out2d[c * M : (c + 1) * M] = outT_c.T
    return out2d.reshape(B, S, D_OUT)
